# revision 34
# baseline (speedup 1.0000x reference)
"""Griffin-Lim phase reconstruction on Trainium2 (Bass/Tile).

Same algorithm as the validated baseline (frame crop + cos/sin phase
carry; see kernel_baseline.py), restructured to cut per-iteration serial
latency.  Measured: 375.9us (baseline) -> ~218us (this kernel), rel err
2.9e-5 vs the 2e-2 gate.

  * TC=64 frame crop (was 80): influence on the returned first 1000
    samples decays with distance; host emulation shows 1.7e-5 rel err at
    TC=64 (cliff starts below ~48 frames).
  * ISTFT + overlap-add fused into 16 accumulating matmuls: tap j of the
    OLA is a K=128 -> M=32 matmul (lhsT = a_r[:, 32j:32j+32]) against a
    column-shifted slice of the spec state.  Taps are spread over the 4
    PE column-groups (tile_position=(0,32g), g=j%4) so 4 matmuls run
    CONCURRENTLY in the array (HW col-tiling; serial-16 measured 330us
    vs 232us packed).  The 4 partition-group partial sums fold with 2
    copies (DVE+ACT in parallel) + 3 adds + the 1/win^2 normalize,
    replacing the baseline's serial chain of 7 PSUM-source DVE adds.
  * STFT frame gather as 8 col-packed K=32 identity matmuls + one
    PSUM->SBUF copy per half (DVE ga / ACT gb), replacing 8
    partition-shifted DVE copies (~-15us total).
  * STFT itself as 16 col-packed M=32 matmuls (4 concurrent rounds).
  * Phase update: (t2+eps)^2 via ACT Square straight from PSUM (one op
    per half), |z| via ACT Sqrt, 1/|z| via the single-op
    reciprocal_approx_fast (51 ULP) instead of the iterative reciprocal,
    spec update via two scalar_tensor_tensor ops reading t2 directly.
  * DC/Nyquist rows (sa/sb row 0) via ACT Sign (same LUT table set as
    Sqrt/Square/Copy -> no table switches).
  * fp32 everywhere: quantizing the ISTFT weights to 16-bit moves the
    Griffin-Lim fixed point past the error budget (fp16 weights alone
    give 2.8e-2 in host emulation); 16-bit buys little here anyway since
    LDWEIGHTS and fixed op overheads dominate at these tile sizes.

PSUM layout constraints that shaped this (see memory notes): a DVE/ACT op
may read only ONE non-scalar PSUM input; PSUM reads must be 32-partition
aligned; an accumulation group's start=True clears its bank's whole 2KB
zero region, so groups may share a bank only on disjoint partition
ranges (wavps, t2r, t2i col-splits), never at different byte offsets.
"""

import numpy as np
from contextlib import ExitStack

import concourse.bass as bass
import concourse.tile as tile
from concourse import bacc, mybir
from concourse import bass_utils

F32 = mybir.dt.float32
AF = mybir.ActivationFunctionType
OP = mybir.AluOpType

TC = 64           # cropped frame count (of 1000)
TS = TC - 7       # stft / phase-update frame count
PAD = 7
N_ITER = 32
N_FFT = 256
NF = 129
HOP = 32
N_CORES = 8
B = 4


def _consts():
    n = np.arange(N_FFT, dtype=np.float64)
    win = 0.5 - 0.5 * np.cos(2.0 * np.pi * n / N_FFT)
    k = np.arange(128, dtype=np.float64)[:, None]
    ang = 2.0 * np.pi * k * n[None, :] / N_FFT
    ck = np.where(k == 0, 1.0, 2.0) / N_FFT
    a_r = (ck * np.cos(ang) * win[None, :]).astype(np.float32)       # (128,256)
    a_i = (-2.0 / N_FFT * np.sin(ang) * win[None, :]).astype(np.float32)
    a_i[0] = (np.cos(np.pi * n) / N_FFT * win).astype(np.float32)    # Nyquist row
    f = np.arange(128, dtype=np.float64)[None, :]
    ang2 = 2.0 * np.pi * f * n[:, None] / N_FFT                      # (256,128)
    bc = (win[:, None] * np.cos(ang2)).astype(np.float32)
    bi = (-win[:, None] * np.sin(ang2)).astype(np.float32)
    bi[:, 0] = (win * np.cos(np.pi * n)).astype(np.float32)
    L = TC * HOP
    wsq = np.zeros((TC + 8) * HOP + N_FFT, dtype=np.float64)
    w2 = win ** 2
    for t in range(TC + 8):
        wsq[t * HOP:t * HOP + N_FFT] += w2
    wsq = np.maximum(wsq[:L], 1e-8)
    invwsq = (1.0 / wsq).astype(np.float32).reshape(TC, HOP).T.copy()  # (32, TC)
    return a_r, a_i, bc.copy(), bi.copy(), invwsq


def _emit(tc_ctx, aps, rep=1):
    tc = tc_ctx
    nc = tc.nc
    with ExitStack() as ctx:
        consts = ctx.enter_context(tc.tile_pool(name="consts", bufs=1))
        state = ctx.enter_context(tc.tile_pool(name="state", bufs=1))
        work = ctx.enter_context(tc.tile_pool(name="work", bufs=3))
        psum = ctx.enter_context(tc.tile_pool(name="psum", bufs=2, space="PSUM"))
        psg = ctx.enter_context(tc.tile_pool(name="psg", bufs=1, space="PSUM"))

        a_r = consts.tile([128, 256], F32)
        a_i = consts.tile([128, 256], F32)
        bca = consts.tile([128, 128], F32)
        bcb = consts.tile([128, 128], F32)
        bia = consts.tile([128, 128], F32)
        bib = consts.tile([128, 128], F32)
        invw = consts.tile([32, TC], F32)
        ident = consts.tile([32, 32], F32)
        maga = consts.tile([128, TS], F32)
        magrow = consts.tile([1, 2 * TS], F32)
        sa = state.tile([128, TC + 2 * PAD], F32)
        sb = state.tile([128, TC + 2 * PAD], F32)
        epsb = consts.tile([128, 1], F32)
        nc.vector.memset(epsb, 1e-6)

        for t, name in [(a_r, "a_r"), (a_i, "a_i"), (bca, "bca"), (bcb, "bcb"),
                        (bia, "bia"), (bib, "bib"),
                        (invw, "invw"), (ident, "ident"),
                        (maga, "maga"), (magrow, "magrow")]:
            nc.sync.dma_start(out=t, in_=aps[name])

        if rep > 1:
            from concourse.engine_type import EngineType
            loop = tc.For_i(0, rep, 1, hint_engines=(
                EngineType.PE, EngineType.DVE, EngineType.Activation,
                EngineType.SP))
        else:
            loop = None
        if loop is not None:
            loop.__enter__()
        nc.sync.dma_start(out=sa, in_=aps["sa0"])
        nc.sync.dma_start(out=sb, in_=aps["sb0"])

        for it in range(N_ITER):
            last = it == N_ITER - 1
            # ---- ISTFT + OLA: 16 col-packed accumulating matmuls ----
            # tap j of the OLA is a K=128 -> M=32 matmul against a column-
            # shifted slice of the spec state.  Taps are spread over the 4
            # PE column-groups (tile_position=(0,32g), g=j%4) so 4 matmuls
            # run CONCURRENTLY in the array (hardware col-tiling; the sim
            # cost model doesn't capture this, HW measures ~2-3x).
            wavps = psum.tile([128, TC], F32, tag="wav")
            for src, w, first in ((sa, a_r, True), (sb, a_i, False)):
                for half in (0, 4):
                    for g in range(4):
                        j = g + half
                        st = first and half == 0
                        # tap j contributes nothing to output columns < j,
                        # but a group's start matmul must write the FULL
                        # range (clears has_written); it reads pad zeros.
                        lo = 0 if st else j
                        nc.tensor.matmul(
                            wavps[32 * g:32 * g + 32, lo:TC],
                            w[:, 32 * j:32 * j + 32],
                            src[:, PAD - j + lo:PAD - j + TC],
                            start=st,
                            stop=((not first) and half == 4),
                            tile_position=(0, 32 * g),
                            skip_group_check=True,
                        )

            if last:
                e1 = work.tile([32, 32], F32, tag="e1")
                e2 = work.tile([32, 32], F32, tag="e2")
                o32 = work.tile([32, 32], F32, tag="o32")
                nc.vector.tensor_copy(e1, wavps[0:32, 0:32])
                nc.scalar.copy(e2, wavps[32:64, 0:32])
                nc.vector.tensor_add(e1, e1, wavps[64:96, 0:32])
                nc.vector.tensor_add(e2, e2, wavps[96:128, 0:32])
                nc.vector.tensor_add(e1, e1, e2)
                nc.vector.tensor_mul(o32, e1, invw[:, 0:32])
                nc.sync.dma_start(out=aps["out"], in_=o32)
                break

            # ---- fold partition groups + window normalize ----
            # (DVE may read only ONE non-scalar PSUM input per op, so the
            #  4-group fold is: 2 parallel copies (DVE+ACT), 2 SBUF+PSUM
            #  adds, combine, normalize)
            q1 = work.tile([32, TC], F32, tag="q1")
            q2 = work.tile([32, TC], F32, tag="q2")
            wn = work.tile([32, TC], F32, tag="wn")
            nc.vector.tensor_copy(q1, wavps[0:32, :])
            nc.scalar.copy(q2, wavps[32:64, :])
            nc.vector.tensor_add(q1, q1, wavps[64:96, :])
            nc.vector.tensor_add(q2, q2, wavps[96:128, :])
            nc.vector.tensor_add(q1, q1, q2)
            nc.vector.tensor_mul(wn, q1, invw)

            # ---- STFT frame gather: 8 col-packed identity matmuls ----
            # ga[32g+i, m] = wn[i, m+g] via K=32 identity matmuls into the
            # 4 PE column-groups (concurrent on HW), then ONE PSUM->SBUF
            # copy per half instead of 4 partition-shifted DVE copies.
            gaps = psg.tile([128, TS], F32, tag="gaps")
            gbps = psg.tile([128, TS], F32, tag="gbps")
            for g in range(4):
                nc.tensor.matmul(gaps[32 * g:32 * g + 32, :], ident,
                                 wn[:, g:g + TS], start=True, stop=True,
                                 tile_position=(0, 32 * g),
                                 skip_group_check=True)
            for g in range(4):
                nc.tensor.matmul(gbps[32 * g:32 * g + 32, :], ident,
                                 wn[:, 4 + g:4 + g + TS], start=True,
                                 stop=True, tile_position=(0, 32 * g),
                                 skip_group_check=True)
            ga = work.tile([128, TS], F32, tag="ga")
            gb = work.tile([128, TS], F32, tag="gb")
            nc.vector.tensor_copy(ga, gaps)
            nc.scalar.copy(gb, gbps)
            # t2r/t2i must be SEPARATE psum tiles: an accumulation group's
            # start=True clears the whole 2KB zero region of its bank, so
            # two groups can never share a bank at different byte offsets.
            # (Groups on DIFFERENT partition ranges of one bank are fine --
            # that's how the col-packed splits below accumulate.)
            # Each 128-wide matmul is split into 4 col-packed M=32 matmuls
            # (tile_position=(0,32g)) so they run concurrently in the array;
            # LDWEIGHTS time is unchanged (4x P=32 = 1x P=128).
            t2r = psum.tile([128, TS], F32, tag="t2r")
            t2i = psum.tile([128, TS], F32, tag="t2i")
            for out, wga, wgb in ((t2r, bca, bcb), (t2i, bia, bib)):
                for g in range(4):
                    nc.tensor.matmul(out[32 * g:32 * g + 32, :],
                                     wga[:, 32 * g:32 * g + 32], ga,
                                     start=True, stop=False,
                                     tile_position=(0, 32 * g),
                                     skip_group_check=True)
            for out, wga, wgb in ((t2r, bca, bcb), (t2i, bia, bib)):
                for g in range(4):
                    nc.tensor.matmul(out[32 * g:32 * g + 32, :],
                                     wgb[:, 32 * g:32 * g + 32], gb,
                                     start=False, stop=True,
                                     tile_position=(0, 32 * g),
                                     skip_group_check=True)

            # ---- phase update: z/|z| carried as (cos, sin) ----
            # ACT functions used: Square, Sqrt, Sign, Copy -- all in the
            # sqrt_and_others LUT set, so no table switches.
            sq2r = work.tile([128, TS], F32, tag="sq2r")
            sq2i = work.tile([128, TS], F32, tag="sq2i")
            nc.scalar.activation(sq2r, t2r, AF.Square, bias=epsb)
            nc.scalar.activation(sq2i, t2i, AF.Square, bias=epsb)
            sq = work.tile([128, TS], F32, tag="sq")
            nc.vector.tensor_add(sq, sq2r, sq2i)
            hyp = work.tile([128, TS], F32, tag="hyp")
            nc.scalar.activation(hyp, sq, AF.Sqrt)
            sgr = work.tile([1, TS], F32, tag="sgr")
            sgi = work.tile([1, TS], F32, tag="sgi")
            nc.scalar.activation(sgr, t2r[0:1, :], AF.Sign, bias=epsb[0:1, :])
            nc.scalar.activation(sgi, t2i[0:1, :], AF.Sign, bias=epsb[0:1, :])
            inv = work.tile([128, TS], F32, tag="inv")
            nc.vector.reciprocal_approx_fast(out=inv, in_=hyp)
            pm = work.tile([128, TS], F32, tag="pm")
            nc.vector.tensor_mul(pm, maga, inv)
            # sa/sb = (t2 + eps) * pm  (row 0 is overwritten below: PSUM
            # reads must be 32-partition aligned, so compute all 128 rows).
            # sa completes first (incl row 0) so the next iteration's
            # sa-tap matmuls can start while sb is still being written.
            nc.vector.scalar_tensor_tensor(
                sa[:, PAD:PAD + TS], t2r, 1e-6, pm, OP.add, OP.mult)
            nc.vector.scalar_tensor_tensor(
                sb[:, PAD:PAD + TS], t2i, 1e-6, pm, OP.add, OP.mult)
            nc.vector.tensor_mul(sa[0:1, PAD:PAD + TS], sgr,
                                 magrow[:, 0:TS])
            nc.vector.tensor_mul(sb[0:1, PAD:PAD + TS], sgi,
                                 magrow[:, TS:2 * TS])
        if loop is not None:
            loop.__exit__(None, None, None)


_CACHED = None


def _build(rep=1):
    global _CACHED
    if rep == 1 and _CACHED is not None:
        return _CACHED
    nc = bacc.Bacc("TRN2", target_bir_lowering=False, debug=False,
                   num_devices=N_CORES)
    shapes = {
        "a_r": (128, 256), "a_i": (128, 256), "bca": (128, 128),
        "bcb": (128, 128), "bia": (128, 128), "bib": (128, 128),
        "invw": (32, TC), "ident": (32, 32), "maga": (128, TS), "magrow": (1, 2 * TS),
        "sa0": (128, TC + 2 * PAD), "sb0": (128, TC + 2 * PAD),
    }
    aps = {name: nc.dram_tensor(name, shape, F32, kind="ExternalInput").ap()
           for name, shape in shapes.items()}
    aps["out"] = nc.dram_tensor("out", (32, 32), F32, kind="ExternalOutput").ap()
    with tile.TileContext(nc) as t:
        _emit(t, aps, rep=rep)
    nc.compile()
    if rep == 1:
        _CACHED = nc
    return nc


def _host_inputs(mag_b, ph_b):
    """Per-batch host prep: crop, initial cos/sin spec chunks, padding."""
    a_r, a_i, bc, bi, invwsq = _consts()
    mag = np.ascontiguousarray(mag_b[:, :TC]).astype(np.float32)
    ph = np.ascontiguousarray(ph_b[:, :TC]).astype(np.float32)
    sa0 = np.zeros((128, TC + 2 * PAD), np.float32)
    sb0 = np.zeros((128, TC + 2 * PAD), np.float32)
    sa0[:, PAD:PAD + TC] = mag[0:128] * np.cos(ph[0:128])
    sb0[0, PAD:PAD + TC] = mag[128] * np.cos(ph[128])
    sb0[1:, PAD:PAD + TC] = mag[1:128] * np.sin(ph[1:128])
    magrow = np.concatenate([mag[0:1, :TS], mag[128:129, :TS]], axis=1)
    return {
        "a_r": a_r, "a_i": a_i,
        "bca": np.ascontiguousarray(bc[0:128]), "bcb": np.ascontiguousarray(bc[128:256]),
        "bia": np.ascontiguousarray(bi[0:128]), "bib": np.ascontiguousarray(bi[128:256]),
        "invw": invwsq,
        "ident": np.eye(32, dtype=np.float32),
        "maga": np.ascontiguousarray(mag[0:128, :TS]),
        "magrow": np.ascontiguousarray(magrow),
        "sa0": sa0, "sb0": sb0,
    }


def kernel(mag_spec, phase):
    mag_spec = np.asarray(mag_spec, dtype=np.float32)
    phase = np.asarray(phase, dtype=np.float32)
    nc = _build()
    in_maps = [_host_inputs(mag_spec[c % B], phase[c % B]) for c in range(N_CORES)]
    res = bass_utils.run_bass_kernel_spmd(nc, in_maps, core_ids=list(range(N_CORES)))
    out = np.zeros((B, 1000), np.float32)
    for b in range(B):
        blk = res.results[b]["out"]              # (32, 32): [i, m] = wav[32m+i]
        out[b] = blk.T.reshape(-1)[15:1015]
    return out
